# revision 22
# baseline (speedup 1.0000x reference)
"""Causal multi-head attention (B=2, T=2048, C=1024, H=16, D=64) on 8 TRN2 cores.

Sharding: 2 heads per core (head-parallel). Per batch, QKV projection and
attention are pipelined at 512-token block granularity: block tb's projection
(tensor-engine work) overlaps the softmax exp (scalar-engine work) of query
tile m=tb-1, keeping both engines busy.

  qkvT = W_slice.T @ xT            per 512-token block
  V_ext[k, h, d] built with XBAR dma transposes (no PE/DVE work)
  scoresT[k, q] = K @ Q.T / 8      both heads adjacent: 64-contract matmuls in
                                   opposite PE row-halves run concurrently
  attT = exp(scoresT) * tri-mask   trimmed to the causal region
  accT[:, h] = [V_h | 1].T @ attT_h  rows 0-63 numerator.T, row 64 denominator
  yT = accT[0:64] * recip(accT[64])
  partial = yT.T @ W_proj_rows     software-pipelined into the next m-tile
Host sums the 8 partials. Matmul inputs bf16, PSUM accumulation fp32.
"""
import sys

sys.path.insert(0, "/opt/trn_rl_repo")

import numpy as np
import ml_dtypes

import concourse.bass as bass
import concourse.mybir as mybir
from concourse import bacc
from concourse.tile import TileContext
from concourse.bass_utils import run_bass_kernel_spmd

N_CORES = 8
B, T, C = 2, 2048, 1024
D = 64          # head dim
NH = 2          # heads per core
HC = NH * D     # 128: head-channels per core
BT = B * T      # 4096
TQ = 512        # query tile
NM = T // TQ    # 4 query tiles per batch
NKB = T // 128  # 16 key blocks per batch
F32 = mybir.dt.float32
BF16 = mybir.dt.bfloat16
NPBF16 = ml_dtypes.bfloat16
SCALE = 1.0 / np.sqrt(D)  # 0.125


def build_program(trace_scopes: bool = False):
    nc = bacc.Bacc("TRN2", target_bir_lowering=False, debug=False)

    xT = nc.dram_tensor("xT", [C, BT], BF16, kind="ExternalInput")
    wk = nc.dram_tensor("wk", [C, 3 * HC], BF16, kind="ExternalInput")
    wp = nc.dram_tensor("wp", [HC, C], BF16, kind="ExternalInput")
    zt = nc.dram_tensor("zt", [128, 4, 128], BF16, kind="ExternalInput")
    ident = nc.dram_tensor("ident", [128, 128], BF16, kind="ExternalInput")
    po = nc.dram_tensor("po", [BT, C], F32, kind="ExternalOutput")

    xT3 = xT.ap().rearrange("(o p) t -> p o t", p=128)   # [128, 8, 4096]
    wk3 = wk.ap().rearrange("(o p) m -> p o m", p=128)   # [128, 8, 384]
    po4 = po.ap().rearrange("(x t4 p) c -> p x t4 c", p=128, t4=4)  # [128, 8, 4, C]

    with TileContext(nc) as tc:
        with (
            tc.tile_pool(name="consts", bufs=1) as consts,
            tc.tile_pool(name="xin", bufs=1) as xin,
            tc.tile_pool(name="qkv", bufs=1) as qkvp,
            tc.tile_pool(name="vext", bufs=1) as vextp,
            tc.tile_pool(name="att", bufs=4) as attp,
            tc.tile_pool(name="yt", bufs=1) as ytp,
            tc.tile_pool(name="oout", bufs=2) as outp,
            tc.tile_pool(name="nrm", bufs=2) as nrmp,
            tc.tile_pool(name="pw", bufs=2, space="PSUM") as pwps,
            tc.tile_pool(name="acc", bufs=1, space="PSUM") as accps,
            tc.tile_pool(name="shr", bufs=2, space="PSUM") as shrps,
        ):
            wq_sb = consts.tile([128, 8, 3 * HC], BF16)
            nc.sync.dma_start(wq_sb[:], wk3)
            wp_sb = consts.tile([HC, C], BF16)
            zt_sb = consts.tile([128, 4, 128], BF16)
            id_sb = consts.tile([128, 128], BF16)
            ones_sb = consts.tile([128, 1], BF16)
            nc.gpsimd.memset(ones_sb[:], 1.0)

            # Prefetch all x blocks (both batches) + consts up front, split
            # across the two DMA-capable queues (sync + scalar) so transfers
            # start in parallel from t=0.
            xblks = []
            for b in range(B):
                for tb in range(4):
                    xblk = xin.tile([128, 8, TQ], BF16, tag=f"xblk{b}{tb}")
                    c0 = b * T + tb * TQ
                    eng = nc.sync if b * 4 + tb < 2 else nc.scalar
                    eng.dma_start(xblk[:], xT3[:, :, c0 : c0 + TQ])
                    xblks.append(xblk)
                if b == 0:
                    nc.sync.dma_start(wp_sb[:], wp.ap())
                    nc.sync.dma_start(zt_sb[:], zt.ap())
                    nc.sync.dma_start(id_sb[:], ident.ap())

            pending_proj = [None]

            def make_proj(x8, yT_):
                q0_ = (x8 % 4) * TQ

                def proj():
                    ot = outp.tile([128, 4, C], F32, tag="ot")
                    for t4 in range(4):
                        t0 = q0_ + t4 * 128
                        for nn in range(2):
                            pj = shrps.tile([128, TQ], F32, tag="s")
                            nc.tensor.matmul(
                                pj[:],
                                yT_[:, t0 : t0 + 128],
                                wp_sb[:, nn * TQ : (nn + 1) * TQ],
                                start=True,
                                stop=True,
                            )
                            nc.vector.tensor_copy(
                                ot[:, t4, nn * TQ : (nn + 1) * TQ], pj[:]
                            )
                        # drain each 128-token block as soon as it's copied
                        nc.sync.dma_start(po4[:, x8, t4], ot[:, t4])

                return proj

            for b in range(B):
                QT = qkvp.tile([128, T], BF16, tag="QT")
                KT = qkvp.tile([128, T], BF16, tag="KT")
                VT = qkvp.tile([128, T], BF16, tag="VT")
                dsts = [QT, KT, VT]
                V_ext = vextp.tile([128, NH, NKB, D + 1], BF16, tag="vext")
                nc.vector.tensor_copy(
                    V_ext[:, :, :, D : D + 1],
                    ones_sb[:].to_broadcast((128, NH, NKB, 1)),
                )
                for tb in range(4):
                    # ---------- QKV projection for 512-token block tb ----------
                    xblk = xblks[b * 4 + tb]
                    for mt in range(3):
                        ps = shrps.tile([128, TQ], F32, tag="s")
                        for kt in range(8):
                            nc.tensor.matmul(
                                ps[:],
                                wq_sb[:, kt, mt * 128 : (mt + 1) * 128],
                                xblk[:, kt, :],
                                start=(kt == 0),
                                stop=(kt == 7),
                            )
                        nc.vector.tensor_copy(
                            dsts[mt][:, tb * TQ : (tb + 1) * TQ], ps[:]
                        )
                    # V_ext[key, h, d] for this block: PE transpose + copy
                    for k4 in range(4):
                        kb = tb * 4 + k4
                        vt = shrps.tile([128, 128], BF16, tag="s")
                        nc.tensor.transpose(
                            vt[:], VT[:, kb * 128 : (kb + 1) * 128], id_sb[:]
                        )
                        nc.vector.tensor_copy(
                            V_ext[:, :, kb, 0:D],
                            vt.rearrange("p (h d) -> p h d", h=NH),
                        )

                    # ---------- attention for query tile m = tb ----------
                    m, q0 = tb, tb * TQ
                    acc = accps.tile([D + 1, NH, TQ], F32, tag="acc")
                    nkb_m = 4 * (m + 1)

                    def scores_exp(kb):
                        jj = kb - 4 * m
                        q_lo = max(jj, 0) * 128  # queries < q_lo are masked
                        pw = pwps.tile([128, NH, TQ], F32, tag="pw")
                        for h in range(NH):
                            hs = slice(h * D, (h + 1) * D)
                            # h=0 uses PE rows 0-63, h=1 rows 64-127 (concurrent)
                            nc.tensor.matmul(
                                pw[:, h, q_lo:],
                                KT[hs, kb * 128 : (kb + 1) * 128],
                                QT[hs, q0 + q_lo : q0 + TQ],
                                start=True,
                                stop=True,
                            )
                        attT = attp.tile([128, NH, TQ], BF16, tag="attT")
                        nc.scalar.activation(
                            attT[:, :, q_lo:],
                            pw[:, :, q_lo:],
                            mybir.ActivationFunctionType.Exp,
                            scale=float(SCALE),
                        )
                        if jj >= 0:
                            # triangular mask on the diagonal block
                            ds = slice(jj * 128, (jj + 1) * 128)
                            nc.vector.tensor_tensor(
                                attT[:, :, ds],
                                attT[:, :, ds],
                                zt_sb[:, 3:4, :].to_broadcast((128, NH, 128)),
                                mybir.AluOpType.mult,
                            )
                        return attT, q_lo

                    def attnv(kb, attT, q_lo):
                        for h in range(NH):
                            nc.tensor.matmul(
                                acc[:, h, q_lo:],
                                V_ext[:, h, kb, :],
                                attT[:, h, q_lo:],
                                start=(kb == 0),
                                stop=(kb == nkb_m - 1),
                            )

                    prev = None
                    for kb in range(nkb_m):
                        cur = scores_exp(kb)
                        if prev is not None:
                            attnv(kb - 1, *prev)
                        if kb == 6 and pending_proj[0] is not None:
                            # previous m-tile's out-projection: by now its yT
                            # is ready, and emitting it here keeps it clear of
                            # this m-tile's normalize/mask work on the DVE
                            pending_proj[0]()
                            pending_proj[0] = None
                        prev = cur
                    attnv(nkb_m - 1, *prev)
                    if pending_proj[0] is not None:
                        # short m-tiles (no kb==6): fill the normalize window
                        pending_proj[0]()
                        pending_proj[0] = None

                    # normalize: yT[hs, q-slice] = num / den
                    if tb == 0:
                        yT = ytp.tile([HC, T], BF16, tag="yT")
                    for h in range(NH):
                        hs = slice(h * D, (h + 1) * D)
                        d_sb = nrmp.tile([1, TQ], F32, tag="d")
                        nc.vector.tensor_copy(d_sb[:], acc[D : D + 1, h, :])
                        r = nrmp.tile([1, TQ], F32, tag="r")
                        nc.vector.reciprocal_approx_fast(r[:], d_sb[:])
                        rb = nrmp.tile([D, TQ], F32, tag="rb")
                        nc.gpsimd.partition_broadcast(rb[:], r[:])
                        nc.vector.tensor_tensor(
                            yT[hs, q0 : q0 + TQ],
                            acc[0:D, h, :],
                            rb[:],
                            mybir.AluOpType.mult,
                        )
                    pending_proj[0] = make_proj(b * 4 + m, yT)

            pending_proj[0]()  # final m-tile's projection

    nc.compile()
    return nc


def make_in_maps(x: np.ndarray, w_qkv: np.ndarray, w_proj: np.ndarray):
    x = np.ascontiguousarray(x, dtype=np.float32)
    w_qkv = np.asarray(w_qkv, dtype=np.float32)
    w_proj = np.asarray(w_proj, dtype=np.float32)

    xT = np.ascontiguousarray(x.reshape(BT, C).T).astype(NPBF16)  # [C, BT]
    zt = np.zeros((128, 4, 128), dtype=np.float32)
    zt[:, 3] = np.triu(np.ones((128, 128), dtype=np.float32))
    zt = zt.astype(NPBF16)
    ident = np.eye(128, dtype=np.float32).astype(NPBF16)

    in_maps = []
    for i in range(N_CORES):
        cs = slice(HC * i, HC * (i + 1))
        wk_i = np.ascontiguousarray(
            np.concatenate(
                [w_qkv[:, cs], w_qkv[:, C:][:, cs], w_qkv[:, 2 * C :][:, cs]], axis=1
            )
        ).astype(NPBF16)
        wp_i = np.ascontiguousarray(w_proj[cs, :]).astype(NPBF16)
        in_maps.append(
            {"xT": xT, "wk": wk_i, "wp": wp_i, "zt": zt, "ident": ident}
        )
    return in_maps


_CACHED_NC = None


def kernel(x: np.ndarray, w_qkv: np.ndarray, w_proj: np.ndarray) -> np.ndarray:
    global _CACHED_NC
    if _CACHED_NC is None:
        _CACHED_NC = build_program()
    nc = _CACHED_NC

    in_maps = make_in_maps(x, w_qkv, w_proj)
    res = run_bass_kernel_spmd(nc, in_maps, core_ids=list(range(N_CORES)))
    total = np.zeros((BT, C), dtype=np.float64)
    for i in range(N_CORES):
        total += res.results[i]["po"]
    return total.astype(np.float32).reshape(B, T, C)


if __name__ == "__main__":
    rng = np.random.default_rng(0)
    x = rng.standard_normal((B, T, C), dtype=np.float32)
    w_qkv = rng.standard_normal((C, 3 * C), dtype=np.float32) / np.sqrt(C)
    w_proj = rng.standard_normal((C, C), dtype=np.float32) / np.sqrt(C)
    out = kernel(x=x, w_qkv=w_qkv, w_proj=w_proj)
    print(out.shape, out.dtype, np.abs(out).mean())


# revision 24
# speedup vs baseline: 1.0054x; 1.0054x over previous
"""Causal multi-head attention (B=2, T=2048, C=1024, H=16, D=64) on 8 TRN2 cores.

Sharding: 2 heads per core (head-parallel). Per batch, QKV projection and
attention are pipelined at 512-token block granularity: block tb's projection
(tensor-engine work) overlaps the softmax exp (scalar-engine work) of query
tile m=tb-1, keeping both engines busy.

  qkvT = W_slice.T @ xT            per 512-token block
  V_ext[k, h, d] built with XBAR dma transposes (no PE/DVE work)
  scoresT[k, q] = K @ Q.T / 8      both heads adjacent: 64-contract matmuls in
                                   opposite PE row-halves run concurrently
  attT = exp(scoresT) * tri-mask   trimmed to the causal region
  accT[:, h] = [V_h | 1].T @ attT_h  rows 0-63 numerator.T, row 64 denominator
  yT = accT[0:64] * recip(accT[64])
  partial = yT.T @ W_proj_rows     software-pipelined into the next m-tile
Host sums the 8 partials. Matmul inputs bf16, PSUM accumulation fp32.
"""
import sys

sys.path.insert(0, "/opt/trn_rl_repo")

import numpy as np
import ml_dtypes

import concourse.bass as bass
import concourse.mybir as mybir
from concourse import bacc
from concourse.tile import TileContext
from concourse.bass_utils import run_bass_kernel_spmd

N_CORES = 8
B, T, C = 2, 2048, 1024
D = 64          # head dim
NH = 2          # heads per core
HC = NH * D     # 128: head-channels per core
BT = B * T      # 4096
TQ = 512        # query tile
NM = T // TQ    # 4 query tiles per batch
NKB = T // 128  # 16 key blocks per batch
F32 = mybir.dt.float32
BF16 = mybir.dt.bfloat16
NPBF16 = ml_dtypes.bfloat16
SCALE = 1.0 / np.sqrt(D)  # 0.125


def build_program(trace_scopes: bool = False):
    nc = bacc.Bacc("TRN2", target_bir_lowering=False, debug=False)

    xT = nc.dram_tensor("xT", [C, BT], BF16, kind="ExternalInput")
    wk = nc.dram_tensor("wk", [C, 3 * HC], BF16, kind="ExternalInput")
    wp = nc.dram_tensor("wp", [HC, C], BF16, kind="ExternalInput")
    zt = nc.dram_tensor("zt", [128, 4, 128], BF16, kind="ExternalInput")
    ident = nc.dram_tensor("ident", [128, 128], BF16, kind="ExternalInput")
    po = nc.dram_tensor("po", [BT, C], F32, kind="ExternalOutput")

    xT3 = xT.ap().rearrange("(o p) t -> p o t", p=128)   # [128, 8, 4096]
    wk3 = wk.ap().rearrange("(o p) m -> p o m", p=128)   # [128, 8, 384]
    po4 = po.ap().rearrange("(x t4 p) c -> p x t4 c", p=128, t4=4)  # [128, 8, 4, C]

    with TileContext(nc) as tc:
        with (
            tc.tile_pool(name="consts", bufs=1) as consts,
            tc.tile_pool(name="xin", bufs=1) as xin,
            tc.tile_pool(name="qkv", bufs=1) as qkvp,
            tc.tile_pool(name="vext", bufs=1) as vextp,
            tc.tile_pool(name="att", bufs=4) as attp,
            tc.tile_pool(name="yt", bufs=1) as ytp,
            tc.tile_pool(name="oout", bufs=2) as outp,
            tc.tile_pool(name="nrm", bufs=2) as nrmp,
            tc.tile_pool(name="pw", bufs=2, space="PSUM") as pwps,
            tc.tile_pool(name="acc", bufs=1, space="PSUM") as accps,
            tc.tile_pool(name="shr", bufs=2, space="PSUM") as shrps,
        ):
            wq_sb = consts.tile([128, 8, 3 * HC], BF16)
            nc.sync.dma_start(wq_sb[:], wk3)
            wp_sb = consts.tile([HC, C], BF16)
            zt_sb = consts.tile([128, 4, 128], BF16)
            id_sb = consts.tile([128, 128], BF16)
            ones_sb = consts.tile([128, 1], BF16)
            nc.gpsimd.memset(ones_sb[:], 1.0)

            # Prefetch all x blocks (both batches) + consts up front, split
            # across the two DMA-capable queues (sync + scalar) so transfers
            # start in parallel from t=0.
            xblks = []
            for b in range(B):
                for tb in range(4):
                    xblk = xin.tile([128, 8, TQ], BF16, tag=f"xblk{b}{tb}")
                    c0 = b * T + tb * TQ
                    if b == 0 and tb == 0:
                        # split the critical first block so its leading half
                        # lands (and the first matmuls start) sooner
                        nc.sync.dma_start(xblk[:, 0:4, :], xT3[:, 0:4, c0 : c0 + TQ])
                        nc.sync.dma_start(xblk[:, 4:8, :], xT3[:, 4:8, c0 : c0 + TQ])
                    else:
                        nc.sync.dma_start(xblk[:], xT3[:, :, c0 : c0 + TQ])
                    xblks.append(xblk)
                if b == 0:
                    nc.sync.dma_start(wp_sb[:], wp.ap())
                    nc.sync.dma_start(zt_sb[:], zt.ap())
                    nc.sync.dma_start(id_sb[:], ident.ap())

            pending_proj = [None]

            def make_proj(x8, yT_):
                q0_ = (x8 % 4) * TQ

                def proj():
                    ot = outp.tile([128, 4, C], F32, tag="ot")
                    for t4 in range(4):
                        t0 = q0_ + t4 * 128
                        for nn in range(2):
                            pj = shrps.tile([128, TQ], F32, tag="s")
                            nc.tensor.matmul(
                                pj[:],
                                yT_[:, t0 : t0 + 128],
                                wp_sb[:, nn * TQ : (nn + 1) * TQ],
                                start=True,
                                stop=True,
                            )
                            nc.vector.tensor_copy(
                                ot[:, t4, nn * TQ : (nn + 1) * TQ], pj[:]
                            )
                        # drain each 128-token block as soon as it's copied;
                        # alternate queues so transfers overlap
                        eng = nc.sync if t4 % 2 == 0 else nc.gpsimd
                        eng.dma_start(po4[:, x8, t4], ot[:, t4])

                return proj

            for b in range(B):
                QT = qkvp.tile([128, T], BF16, tag="QT")
                KT = qkvp.tile([128, T], BF16, tag="KT")
                VT = qkvp.tile([128, T], BF16, tag="VT")
                dsts = [QT, KT, VT]
                V_ext = vextp.tile([128, NH, NKB, D + 1], BF16, tag="vext")
                nc.vector.tensor_copy(
                    V_ext[:, :, :, D : D + 1],
                    ones_sb[:].to_broadcast((128, NH, NKB, 1)),
                )
                for tb in range(4):
                    # ---------- QKV projection for 512-token block tb ----------
                    xblk = xblks[b * 4 + tb]
                    for mt in range(3):
                        ps = shrps.tile([128, TQ], F32, tag="s")
                        for kt in range(8):
                            nc.tensor.matmul(
                                ps[:],
                                wq_sb[:, kt, mt * 128 : (mt + 1) * 128],
                                xblk[:, kt, :],
                                start=(kt == 0),
                                stop=(kt == 7),
                            )
                        nc.vector.tensor_copy(
                            dsts[mt][:, tb * TQ : (tb + 1) * TQ], ps[:]
                        )
                    # V_ext[key, h, d] for this block: PE transpose + copy
                    for k4 in range(4):
                        kb = tb * 4 + k4
                        vt = shrps.tile([128, 128], BF16, tag="s")
                        nc.tensor.transpose(
                            vt[:], VT[:, kb * 128 : (kb + 1) * 128], id_sb[:]
                        )
                        nc.vector.tensor_copy(
                            V_ext[:, :, kb, 0:D],
                            vt.rearrange("p (h d) -> p h d", h=NH),
                        )

                    # ---------- attention for query tile m = tb ----------
                    m, q0 = tb, tb * TQ
                    acc = accps.tile([D + 1, NH, TQ], F32, tag="acc")
                    nkb_m = 4 * (m + 1)

                    def scores_exp(kb):
                        jj = kb - 4 * m
                        q_lo = max(jj, 0) * 128  # queries < q_lo are masked
                        pw = pwps.tile([128, NH, TQ], F32, tag="pw")
                        for h in range(NH):
                            hs = slice(h * D, (h + 1) * D)
                            # h=0 uses PE rows 0-63, h=1 rows 64-127 (concurrent)
                            nc.tensor.matmul(
                                pw[:, h, q_lo:],
                                KT[hs, kb * 128 : (kb + 1) * 128],
                                QT[hs, q0 + q_lo : q0 + TQ],
                                start=True,
                                stop=True,
                            )
                        attT = attp.tile([128, NH, TQ], BF16, tag="attT")
                        nc.scalar.activation(
                            attT[:, :, q_lo:],
                            pw[:, :, q_lo:],
                            mybir.ActivationFunctionType.Exp,
                            scale=float(SCALE),
                        )
                        if jj >= 0:
                            # triangular mask on the diagonal block
                            ds = slice(jj * 128, (jj + 1) * 128)
                            nc.vector.tensor_tensor(
                                attT[:, :, ds],
                                attT[:, :, ds],
                                zt_sb[:, 3:4, :].to_broadcast((128, NH, 128)),
                                mybir.AluOpType.mult,
                            )
                        return attT, q_lo

                    def attnv(kb, attT, q_lo):
                        for h in range(NH):
                            nc.tensor.matmul(
                                acc[:, h, q_lo:],
                                V_ext[:, h, kb, :],
                                attT[:, h, q_lo:],
                                start=(kb == 0),
                                stop=(kb == nkb_m - 1),
                            )

                    prev = None
                    for kb in range(nkb_m):
                        cur = scores_exp(kb)
                        if prev is not None:
                            attnv(kb - 1, *prev)
                        if kb == 6 and pending_proj[0] is not None:
                            # previous m-tile's out-projection: by now its yT
                            # is ready, and emitting it here keeps it clear of
                            # this m-tile's normalize/mask work on the DVE
                            pending_proj[0]()
                            pending_proj[0] = None
                        prev = cur
                    attnv(nkb_m - 1, *prev)
                    if pending_proj[0] is not None:
                        # short m-tiles (no kb==6): fill the normalize window
                        pending_proj[0]()
                        pending_proj[0] = None

                    # normalize: yT[hs, q-slice] = num / den
                    if tb == 0:
                        yT = ytp.tile([HC, T], BF16, tag="yT")
                    for h in range(NH):
                        hs = slice(h * D, (h + 1) * D)
                        d_sb = nrmp.tile([1, TQ], F32, tag="d")
                        nc.vector.tensor_copy(d_sb[:], acc[D : D + 1, h, :])
                        r = nrmp.tile([1, TQ], F32, tag="r")
                        nc.vector.reciprocal_approx_fast(r[:], d_sb[:])
                        rb = nrmp.tile([D, TQ], F32, tag="rb")
                        nc.gpsimd.partition_broadcast(rb[:], r[:])
                        nc.vector.tensor_tensor(
                            yT[hs, q0 : q0 + TQ],
                            acc[0:D, h, :],
                            rb[:],
                            mybir.AluOpType.mult,
                        )
                    pending_proj[0] = make_proj(b * 4 + m, yT)

            pending_proj[0]()  # final m-tile's projection

    nc.compile()
    return nc


def make_in_maps(x: np.ndarray, w_qkv: np.ndarray, w_proj: np.ndarray):
    x = np.ascontiguousarray(x, dtype=np.float32)
    w_qkv = np.asarray(w_qkv, dtype=np.float32)
    w_proj = np.asarray(w_proj, dtype=np.float32)

    xT = np.ascontiguousarray(x.reshape(BT, C).T).astype(NPBF16)  # [C, BT]
    zt = np.zeros((128, 4, 128), dtype=np.float32)
    zt[:, 3] = np.triu(np.ones((128, 128), dtype=np.float32))
    zt = zt.astype(NPBF16)
    ident = np.eye(128, dtype=np.float32).astype(NPBF16)

    in_maps = []
    for i in range(N_CORES):
        cs = slice(HC * i, HC * (i + 1))
        wk_i = np.ascontiguousarray(
            np.concatenate(
                [w_qkv[:, cs], w_qkv[:, C:][:, cs], w_qkv[:, 2 * C :][:, cs]], axis=1
            )
        ).astype(NPBF16)
        wp_i = np.ascontiguousarray(w_proj[cs, :]).astype(NPBF16)
        in_maps.append(
            {"xT": xT, "wk": wk_i, "wp": wp_i, "zt": zt, "ident": ident}
        )
    return in_maps


_CACHED_NC = None


def kernel(x: np.ndarray, w_qkv: np.ndarray, w_proj: np.ndarray) -> np.ndarray:
    global _CACHED_NC
    if _CACHED_NC is None:
        _CACHED_NC = build_program()
    nc = _CACHED_NC

    in_maps = make_in_maps(x, w_qkv, w_proj)
    res = run_bass_kernel_spmd(nc, in_maps, core_ids=list(range(N_CORES)))
    total = np.zeros((BT, C), dtype=np.float64)
    for i in range(N_CORES):
        total += res.results[i]["po"]
    return total.astype(np.float32).reshape(B, T, C)


if __name__ == "__main__":
    rng = np.random.default_rng(0)
    x = rng.standard_normal((B, T, C), dtype=np.float32)
    w_qkv = rng.standard_normal((C, 3 * C), dtype=np.float32) / np.sqrt(C)
    w_proj = rng.standard_normal((C, C), dtype=np.float32) / np.sqrt(C)
    out = kernel(x=x, w_qkv=w_qkv, w_proj=w_proj)
    print(out.shape, out.dtype, np.abs(out).mean())


# revision 29
# speedup vs baseline: 1.1320x; 1.1259x over previous
"""Causal multi-head attention (B=2, T=2048, C=1024, H=16, D=64) on 8 TRN2 cores.

Sharding: 2 heads per core (head-parallel). Per batch, QKV projection and
attention are pipelined at 512-token block granularity: block tb's projection
(tensor-engine work) overlaps the softmax exp (scalar-engine work) of query
tile m=tb-1, keeping both engines busy.

  qkvT = W_slice.T @ xT            per 512-token block
  V_ext[k, h, d] built with XBAR dma transposes (no PE/DVE work)
  scoresT[k, q] = K @ Q.T / 8      both heads adjacent: 64-contract matmuls in
                                   opposite PE row-halves run concurrently
  attT = exp(scoresT) * tri-mask   trimmed to the causal region
  accT[:, h] = [V_h | 1].T @ attT_h  rows 0-63 numerator.T, row 64 denominator
  yT = accT[0:64] * recip(accT[64])
  partial = yT.T @ W_proj_rows     software-pipelined into the next m-tile
Host sums the 8 partials. Matmul inputs bf16, PSUM accumulation fp32.
"""
import sys

sys.path.insert(0, "/opt/trn_rl_repo")

import numpy as np
import ml_dtypes

import concourse.bass as bass
import concourse.mybir as mybir
from concourse import bacc
from concourse.tile import TileContext
from concourse.bass_utils import run_bass_kernel_spmd

N_CORES = 8
B, T, C = 2, 2048, 1024
D = 64          # head dim
NH = 2          # heads per core
HC = NH * D     # 128: head-channels per core
BT = B * T      # 4096
TQ = 512        # query tile
NM = T // TQ    # 4 query tiles per batch
NKB = T // 128  # 16 key blocks per batch
F32 = mybir.dt.float32
BF16 = mybir.dt.bfloat16
NPBF16 = ml_dtypes.bfloat16
SCALE = 1.0 / np.sqrt(D)  # 0.125


def build_program(trace_scopes: bool = False):
    nc = bacc.Bacc("TRN2", target_bir_lowering=False, debug=False)

    xT = nc.dram_tensor("xT", [C, BT], BF16, kind="ExternalInput")
    wk = nc.dram_tensor("wk", [C, 3 * HC], BF16, kind="ExternalInput")
    wp = nc.dram_tensor("wp", [HC, C], BF16, kind="ExternalInput")
    zt = nc.dram_tensor("zt", [128, 4, 128], BF16, kind="ExternalInput")
    ident = nc.dram_tensor("ident", [128, 128], BF16, kind="ExternalInput")
    po = nc.dram_tensor("po", [BT, C], F32, kind="ExternalOutput")

    xT3 = xT.ap().rearrange("(o p) t -> p o t", p=128)   # [128, 8, 4096]
    wk3 = wk.ap().rearrange("(o p) m -> p o m", p=128)   # [128, 8, 384]
    po4 = po.ap().rearrange("(x t4 p) c -> p x t4 c", p=128, t4=4)  # [128, 8, 4, C]

    with TileContext(nc) as tc:
        with (
            tc.tile_pool(name="consts", bufs=1) as consts,
            tc.tile_pool(name="xin", bufs=1) as xin,
            tc.tile_pool(name="qkv", bufs=1) as qkvp,
            tc.tile_pool(name="vext", bufs=1) as vextp,
            tc.tile_pool(name="att", bufs=4) as attp,
            tc.tile_pool(name="yt", bufs=1) as ytp,
            tc.tile_pool(name="oout", bufs=2) as outp,
            tc.tile_pool(name="nrm", bufs=2) as nrmp,
            tc.tile_pool(name="pw", bufs=2, space="PSUM") as pwps,
            tc.tile_pool(name="acc", bufs=1, space="PSUM") as accps,
            tc.tile_pool(name="shr", bufs=2, space="PSUM") as shrps,
        ):
            wq_sb = consts.tile([128, 8, 3 * HC], BF16)
            nc.sync.dma_start(wq_sb[:], wk3)
            wp_sb = consts.tile([HC, C], BF16)
            zt_sb = consts.tile([128, 4, 128], BF16)
            id_sb = consts.tile([128, 128], BF16)
            ones_sb = consts.tile([128, 1], BF16)
            nc.gpsimd.memset(ones_sb[:], 1.0)

            # Prefetch all x blocks (both batches) + consts up front, split
            # across the two DMA-capable queues (sync + scalar) so transfers
            # start in parallel from t=0.
            xblks = []
            for b in range(B):
                for tb in range(4):
                    xblk = xin.tile([128, 8, TQ], BF16, tag=f"xblk{b}{tb}")
                    c0 = b * T + tb * TQ
                    if b == 0 and tb == 0:
                        # split the critical first block so its leading half
                        # lands (and the first matmuls start) sooner
                        nc.sync.dma_start(xblk[:, 0:4, :], xT3[:, 0:4, c0 : c0 + TQ])
                        nc.sync.dma_start(xblk[:, 4:8, :], xT3[:, 4:8, c0 : c0 + TQ])
                    else:
                        nc.sync.dma_start(xblk[:], xT3[:, :, c0 : c0 + TQ])
                    xblks.append(xblk)
                if b == 0:
                    nc.sync.dma_start(wp_sb[:], wp.ap())
                    nc.sync.dma_start(zt_sb[:], zt.ap())
                    nc.sync.dma_start(id_sb[:], ident.ap())

            pending_proj = [None]

            def make_proj(x8, yT_):
                q0_ = (x8 % 4) * TQ

                def proj(last: bool = False):
                    ot = outp.tile([128, 4, C], F32, tag="ot")
                    for t4 in range(4):
                        t0 = q0_ + t4 * 128
                        for nn in range(2):
                            pj = shrps.tile([128, TQ], F32, tag="s")
                            nc.tensor.matmul(
                                pj[:],
                                yT_[:, t0 : t0 + 128],
                                wp_sb[:, nn * TQ : (nn + 1) * TQ],
                                start=True,
                                stop=True,
                            )
                            dst = ot[:, t4, nn * TQ : (nn + 1) * TQ]
                            if last and nn == 1:
                                # drain the final tile 2x as fast: ACT is idle
                                # by now, so split copies across both engines
                                nc.scalar.copy(dst, pj[:])
                            else:
                                nc.vector.tensor_copy(dst, pj[:])
                        # drain each 128-token block as soon as it's copied
                        nc.sync.dma_start(po4[:, x8, t4], ot[:, t4])

                return proj

            for b in range(B):
                QT = qkvp.tile([128, T], BF16, tag="QT")
                KT = qkvp.tile([128, T], BF16, tag="KT")
                VT = qkvp.tile([128, T], BF16, tag="VT")
                dsts = [QT, KT, VT]
                V_ext = vextp.tile([128, NH, NKB, D + 1], BF16, tag="vext")
                nc.vector.tensor_copy(
                    V_ext[:, :, :, D : D + 1],
                    ones_sb[:].to_broadcast((128, NH, NKB, 1)),
                )
                for tb in range(4):
                    # ---------- QKV projection for 512-token block tb ----------
                    xblk = xblks[b * 4 + tb]
                    for mt in range(3):
                        ps = shrps.tile([128, TQ], F32, tag="s")
                        for kt in range(8):
                            nc.tensor.matmul(
                                ps[:],
                                wq_sb[:, kt, mt * 128 : (mt + 1) * 128],
                                xblk[:, kt, :],
                                start=(kt == 0),
                                stop=(kt == 7),
                            )
                        nc.vector.tensor_copy(
                            dsts[mt][:, tb * TQ : (tb + 1) * TQ], ps[:]
                        )
                    # V_ext[key, h, d] for this block: PE transpose + copy
                    for k4 in range(4):
                        kb = tb * 4 + k4
                        vt = shrps.tile([128, 128], BF16, tag="s")
                        nc.tensor.transpose(
                            vt[:], VT[:, kb * 128 : (kb + 1) * 128], id_sb[:]
                        )
                        nc.vector.tensor_copy(
                            V_ext[:, :, kb, 0:D],
                            vt.rearrange("p (h d) -> p h d", h=NH),
                        )

                    # ---------- attention for query tile m = tb ----------
                    m, q0 = tb, tb * TQ
                    acc = accps.tile([D + 1, NH, TQ], F32, tag="acc")
                    nkb_m = 4 * (m + 1)

                    def scores_exp(kb):
                        jj = kb - 4 * m
                        q_lo = max(jj, 0) * 128  # queries < q_lo are masked
                        pw = pwps.tile([128, NH, TQ], F32, tag="pw")
                        for h in range(NH):
                            hs = slice(h * D, (h + 1) * D)
                            # h=0 uses PE rows 0-63, h=1 rows 64-127 (concurrent)
                            nc.tensor.matmul(
                                pw[:, h, q_lo:],
                                KT[hs, kb * 128 : (kb + 1) * 128],
                                QT[hs, q0 + q_lo : q0 + TQ],
                                start=True,
                                stop=True,
                            )
                        attT = attp.tile([128, NH, TQ], BF16, tag="attT")
                        nc.scalar.activation(
                            attT[:, :, q_lo:],
                            pw[:, :, q_lo:],
                            mybir.ActivationFunctionType.Exp,
                            scale=float(SCALE),
                        )
                        if jj >= 0:
                            # triangular mask on the diagonal block
                            ds = slice(jj * 128, (jj + 1) * 128)
                            nc.vector.tensor_tensor(
                                attT[:, :, ds],
                                attT[:, :, ds],
                                zt_sb[:, 3:4, :].to_broadcast((128, NH, 128)),
                                mybir.AluOpType.mult,
                            )
                        return attT, q_lo

                    def attnv(kb, attT, q_lo):
                        for h in range(NH):
                            nc.tensor.matmul(
                                acc[:, h, q_lo:],
                                V_ext[:, h, kb, :],
                                attT[:, h, q_lo:],
                                start=(kb == 0),
                                stop=(kb == nkb_m - 1),
                            )

                    prev = None
                    for kb in range(nkb_m):
                        cur = scores_exp(kb)
                        if prev is not None:
                            attnv(kb - 1, *prev)
                        prev = cur
                    attnv(nkb_m - 1, *prev)
                    if pending_proj[0] is not None:
                        # previous m-tile's out-projection: tensor-engine work
                        # that fills this m-tile's normalize wait window
                        pending_proj[0]()
                        pending_proj[0] = None

                    # normalize: yT[hs, q-slice] = num / den. Both heads'
                    # recip chains are emitted before either yT multiply so
                    # the DVE queue never blocks on a gpsimd broadcast.
                    if tb == 0:
                        yT = ytp.tile([HC, T], BF16, tag="yT")
                    rbs = []
                    for h in range(NH):
                        d_sb = nrmp.tile([1, TQ], F32, tag="d")
                        nc.vector.tensor_copy(d_sb[:], acc[D : D + 1, h, :])
                        r = nrmp.tile([1, TQ], F32, tag="r")
                        nc.vector.reciprocal_approx_fast(r[:], d_sb[:])
                        rb = nrmp.tile([D, TQ], F32, tag="rb")
                        nc.gpsimd.partition_broadcast(rb[:], r[:])
                        rbs.append(rb)
                    for h in range(NH):
                        hs = slice(h * D, (h + 1) * D)
                        nc.vector.tensor_tensor(
                            yT[hs, q0 : q0 + TQ],
                            acc[0:D, h, :],
                            rbs[h][:],
                            mybir.AluOpType.mult,
                        )
                    pending_proj[0] = make_proj(b * 4 + m, yT)

            pending_proj[0](last=True)  # final m-tile's projection

    nc.compile()
    return nc


def make_in_maps(x: np.ndarray, w_qkv: np.ndarray, w_proj: np.ndarray):
    x = np.ascontiguousarray(x, dtype=np.float32)
    w_qkv = np.asarray(w_qkv, dtype=np.float32)
    w_proj = np.asarray(w_proj, dtype=np.float32)

    xT = np.ascontiguousarray(x.reshape(BT, C).T).astype(NPBF16)  # [C, BT]
    zt = np.zeros((128, 4, 128), dtype=np.float32)
    zt[:, 3] = np.triu(np.ones((128, 128), dtype=np.float32))
    zt = zt.astype(NPBF16)
    ident = np.eye(128, dtype=np.float32).astype(NPBF16)

    in_maps = []
    for i in range(N_CORES):
        cs = slice(HC * i, HC * (i + 1))
        wk_i = np.ascontiguousarray(
            np.concatenate(
                [w_qkv[:, cs], w_qkv[:, C:][:, cs], w_qkv[:, 2 * C :][:, cs]], axis=1
            )
        ).astype(NPBF16)
        wp_i = np.ascontiguousarray(w_proj[cs, :]).astype(NPBF16)
        in_maps.append(
            {"xT": xT, "wk": wk_i, "wp": wp_i, "zt": zt, "ident": ident}
        )
    return in_maps


_CACHED_NC = None


def kernel(x: np.ndarray, w_qkv: np.ndarray, w_proj: np.ndarray) -> np.ndarray:
    global _CACHED_NC
    if _CACHED_NC is None:
        _CACHED_NC = build_program()
    nc = _CACHED_NC

    in_maps = make_in_maps(x, w_qkv, w_proj)
    res = run_bass_kernel_spmd(nc, in_maps, core_ids=list(range(N_CORES)))
    total = np.zeros((BT, C), dtype=np.float64)
    for i in range(N_CORES):
        total += res.results[i]["po"]
    return total.astype(np.float32).reshape(B, T, C)


if __name__ == "__main__":
    rng = np.random.default_rng(0)
    x = rng.standard_normal((B, T, C), dtype=np.float32)
    w_qkv = rng.standard_normal((C, 3 * C), dtype=np.float32) / np.sqrt(C)
    w_proj = rng.standard_normal((C, C), dtype=np.float32) / np.sqrt(C)
    out = kernel(x=x, w_qkv=w_qkv, w_proj=w_proj)
    print(out.shape, out.dtype, np.abs(out).mean())


# revision 33
# speedup vs baseline: 1.1574x; 1.0224x over previous
"""Causal multi-head attention (B=2, T=2048, C=1024, H=16, D=64) on 8 TRN2 cores.

Sharding: 2 heads per core (head-parallel). Per batch, QKV projection and
attention are pipelined at 512-token block granularity: block tb's projection
(tensor-engine work) overlaps the softmax exp (scalar-engine work) of query
tile m=tb-1, keeping both engines busy.

  qkvT = W_slice.T @ xT            per 512-token block
  V_ext[k, h, d] built with XBAR dma transposes (no PE/DVE work)
  scoresT[k, q] = K @ Q.T / 8      both heads adjacent: 64-contract matmuls in
                                   opposite PE row-halves run concurrently
  attT = exp(scoresT) * tri-mask   trimmed to the causal region
  accT[:, h] = [V_h | 1].T @ attT_h  rows 0-63 numerator.T, row 64 denominator
  yT = accT[0:64] * recip(accT[64])
  partial = yT.T @ W_proj_rows     software-pipelined into the next m-tile
Host sums the 8 partials. Matmul inputs bf16, PSUM accumulation fp32.
"""
import sys

sys.path.insert(0, "/opt/trn_rl_repo")

import numpy as np
import ml_dtypes

import concourse.bass as bass
import concourse.mybir as mybir
from concourse import bacc
from concourse.tile import TileContext
from concourse.bass_utils import run_bass_kernel_spmd

N_CORES = 8
B, T, C = 2, 2048, 1024
D = 64          # head dim
NH = 2          # heads per core
HC = NH * D     # 128: head-channels per core
BT = B * T      # 4096
TQ = 512        # query tile
NM = T // TQ    # 4 query tiles per batch
NKB = T // 128  # 16 key blocks per batch
F32 = mybir.dt.float32
BF16 = mybir.dt.bfloat16
NPBF16 = ml_dtypes.bfloat16
SCALE = 1.0 / np.sqrt(D)  # 0.125


def build_program(trace_scopes: bool = False):
    nc = bacc.Bacc("TRN2", target_bir_lowering=False, debug=False)

    xT = nc.dram_tensor("xT", [C, BT], BF16, kind="ExternalInput")
    wk = nc.dram_tensor("wk", [C, 3 * HC], BF16, kind="ExternalInput")
    wp = nc.dram_tensor("wp", [HC, C], BF16, kind="ExternalInput")
    zt = nc.dram_tensor("zt", [128, 4, 128], BF16, kind="ExternalInput")
    ident = nc.dram_tensor("ident", [128, 128], BF16, kind="ExternalInput")
    po = nc.dram_tensor("po", [BT, C], BF16, kind="ExternalOutput")

    xT3 = xT.ap().rearrange("(o p) t -> p o t", p=128)   # [128, 8, 4096]
    wk3 = wk.ap().rearrange("(o p) m -> p o m", p=128)   # [128, 8, 384]
    po4 = po.ap().rearrange("(x t4 p) c -> p x t4 c", p=128, t4=4)  # [128, 8, 4, C]

    with TileContext(nc) as tc:
        with (
            tc.tile_pool(name="consts", bufs=1) as consts,
            tc.tile_pool(name="xin", bufs=1) as xin,
            tc.tile_pool(name="qkv", bufs=1) as qkvp,
            tc.tile_pool(name="vext", bufs=1) as vextp,
            tc.tile_pool(name="att", bufs=4) as attp,
            tc.tile_pool(name="yt", bufs=1) as ytp,
            tc.tile_pool(name="oout", bufs=2) as outp,
            tc.tile_pool(name="nrm", bufs=2) as nrmp,
            tc.tile_pool(name="pw", bufs=2, space="PSUM") as pwps,
            tc.tile_pool(name="acc", bufs=1, space="PSUM") as accps,
            tc.tile_pool(name="shr", bufs=2, space="PSUM") as shrps,
        ):
            wq_sb = consts.tile([128, 8, 3 * HC], BF16)
            nc.sync.dma_start(wq_sb[:], wk3)
            wp_sb = consts.tile([HC, C], BF16)
            zt_sb = consts.tile([128, 4, 128], BF16)
            id_sb = consts.tile([128, 128], BF16)
            ones_sb = consts.tile([128, 1], BF16)
            nc.gpsimd.memset(ones_sb[:], 1.0)

            # Prefetch all x blocks (both batches) + consts up front, split
            # across the two DMA-capable queues (sync + scalar) so transfers
            # start in parallel from t=0.
            xblks = []
            for b in range(B):
                for tb in range(4):
                    xblk = xin.tile([128, 8, TQ], BF16, tag=f"xblk{b}{tb}")
                    c0 = b * T + tb * TQ
                    if b == 0 and tb == 0:
                        # split the critical first block so its leading half
                        # lands (and the first matmuls start) sooner
                        nc.sync.dma_start(xblk[:, 0:4, :], xT3[:, 0:4, c0 : c0 + TQ])
                        nc.sync.dma_start(xblk[:, 4:8, :], xT3[:, 4:8, c0 : c0 + TQ])
                        # small consts next: zt/ident gate the first masks and
                        # V transposes, well before xblk1 is needed
                        nc.sync.dma_start(zt_sb[:], zt.ap())
                        nc.sync.dma_start(id_sb[:], ident.ap())
                        nc.sync.dma_start(wp_sb[:], wp.ap())
                    else:
                        nc.sync.dma_start(xblk[:], xT3[:, :, c0 : c0 + TQ])
                    xblks.append(xblk)

            pending_proj = [None]

            def make_proj(x8, yT_):
                q0_ = (x8 % 4) * TQ

                def proj(last: bool = False):
                    ot = outp.tile([128, 4, C], BF16, tag="ot")
                    for t4 in range(4):
                        t0 = q0_ + t4 * 128
                        for nn in range(2):
                            pj = shrps.tile([128, TQ], F32, tag="s")
                            nc.tensor.matmul(
                                pj[:],
                                yT_[:, t0 : t0 + 128],
                                wp_sb[:, nn * TQ : (nn + 1) * TQ],
                                start=True,
                                stop=True,
                            )
                            dst = ot[:, t4, nn * TQ : (nn + 1) * TQ]
                            if last and nn == 1:
                                # drain the final tile 2x as fast: ACT is idle
                                # by now, so split copies across both engines
                                nc.scalar.copy(dst, pj[:])
                            else:
                                nc.vector.tensor_copy(dst, pj[:])
                        # drain each 128-token block as soon as it's copied
                        nc.sync.dma_start(po4[:, x8, t4], ot[:, t4])

                return proj

            for b in range(B):
                QT = qkvp.tile([128, T], BF16, tag="QT")
                KT = qkvp.tile([128, T], BF16, tag="KT")
                VT = qkvp.tile([128, T], BF16, tag="VT")
                dsts = [QT, KT, VT]
                V_ext = vextp.tile([128, NH, NKB, D + 1], BF16, tag="vext")
                nc.vector.tensor_copy(
                    V_ext[:, :, :, D : D + 1],
                    ones_sb[:].to_broadcast((128, NH, NKB, 1)),
                )
                for tb in range(4):
                    # ---------- QKV projection for 512-token block tb ----------
                    xblk = xblks[b * 4 + tb]
                    for mt in range(3):
                        ps = shrps.tile([128, TQ], F32, tag="s")
                        for kt in range(8):
                            nc.tensor.matmul(
                                ps[:],
                                wq_sb[:, kt, mt * 128 : (mt + 1) * 128],
                                xblk[:, kt, :],
                                start=(kt == 0),
                                stop=(kt == 7),
                            )
                        nc.vector.tensor_copy(
                            dsts[mt][:, tb * TQ : (tb + 1) * TQ], ps[:]
                        )
                    # V_ext[key, h, d] for this block: PE transpose + copy
                    for k4 in range(4):
                        kb = tb * 4 + k4
                        vt = shrps.tile([128, 128], BF16, tag="s")
                        nc.tensor.transpose(
                            vt[:], VT[:, kb * 128 : (kb + 1) * 128], id_sb[:]
                        )
                        nc.vector.tensor_copy(
                            V_ext[:, :, kb, 0:D],
                            vt.rearrange("p (h d) -> p h d", h=NH),
                        )

                    # ---------- attention for query tile m = tb ----------
                    m, q0 = tb, tb * TQ
                    acc = accps.tile([D + 1, NH, TQ], F32, tag="acc")
                    nkb_m = 4 * (m + 1)

                    def scores_exp(kb):
                        jj = kb - 4 * m
                        q_lo = max(jj, 0) * 128  # queries < q_lo are masked
                        pw = pwps.tile([128, NH, TQ], F32, tag="pw")
                        for h in range(NH):
                            hs = slice(h * D, (h + 1) * D)
                            # h=0 uses PE rows 0-63, h=1 rows 64-127 (concurrent)
                            nc.tensor.matmul(
                                pw[:, h, q_lo:],
                                KT[hs, kb * 128 : (kb + 1) * 128],
                                QT[hs, q0 + q_lo : q0 + TQ],
                                start=True,
                                stop=True,
                            )
                        attT = attp.tile([128, NH, TQ], BF16, tag="attT")
                        nc.scalar.activation(
                            attT[:, :, q_lo:],
                            pw[:, :, q_lo:],
                            mybir.ActivationFunctionType.Exp,
                            scale=float(SCALE),
                        )
                        if jj >= 0:
                            # triangular mask on the diagonal block
                            ds = slice(jj * 128, (jj + 1) * 128)
                            nc.vector.tensor_tensor(
                                attT[:, :, ds],
                                attT[:, :, ds],
                                zt_sb[:, 3:4, :].to_broadcast((128, NH, 128)),
                                mybir.AluOpType.mult,
                            )
                        return attT, q_lo

                    def attnv(kb, attT, q_lo):
                        for h in range(NH):
                            nc.tensor.matmul(
                                acc[:, h, q_lo:],
                                V_ext[:, h, kb, :],
                                attT[:, h, q_lo:],
                                start=(kb == 0),
                                stop=(kb == nkb_m - 1),
                            )

                    prev = None
                    for kb in range(nkb_m):
                        cur = scores_exp(kb)
                        if prev is not None:
                            attnv(kb - 1, *prev)
                        prev = cur
                    attnv(nkb_m - 1, *prev)
                    if pending_proj[0] is not None:
                        # previous m-tile's out-projection: tensor-engine work
                        # that fills this m-tile's normalize wait window
                        pending_proj[0]()
                        pending_proj[0] = None

                    # normalize: yT[hs, q-slice] = num / den. Both heads'
                    # recip chains are emitted before either yT multiply so
                    # the DVE queue never blocks on a gpsimd broadcast.
                    if tb == 0:
                        yT = ytp.tile([HC, T], BF16, tag="yT")
                    rbs = []
                    for h in range(NH):
                        d_sb = nrmp.tile([1, TQ], F32, tag="d")
                        nc.vector.tensor_copy(d_sb[:], acc[D : D + 1, h, :])
                        r = nrmp.tile([1, TQ], F32, tag="r")
                        nc.vector.reciprocal_approx_fast(r[:], d_sb[:])
                        rb = nrmp.tile([D, TQ], F32, tag="rb")
                        nc.gpsimd.partition_broadcast(rb[:], r[:])
                        rbs.append(rb)
                    for h in range(NH):
                        hs = slice(h * D, (h + 1) * D)
                        nc.vector.tensor_tensor(
                            yT[hs, q0 : q0 + TQ],
                            acc[0:D, h, :],
                            rbs[h][:],
                            mybir.AluOpType.mult,
                        )
                    pending_proj[0] = make_proj(b * 4 + m, yT)

            pending_proj[0](last=True)  # final m-tile's projection

    nc.compile()
    return nc


def make_in_maps(x: np.ndarray, w_qkv: np.ndarray, w_proj: np.ndarray):
    x = np.ascontiguousarray(x, dtype=np.float32)
    w_qkv = np.asarray(w_qkv, dtype=np.float32)
    w_proj = np.asarray(w_proj, dtype=np.float32)

    xT = np.ascontiguousarray(x.reshape(BT, C).T).astype(NPBF16)  # [C, BT]
    zt = np.zeros((128, 4, 128), dtype=np.float32)
    zt[:, 3] = np.triu(np.ones((128, 128), dtype=np.float32))
    zt = zt.astype(NPBF16)
    ident = np.eye(128, dtype=np.float32).astype(NPBF16)

    in_maps = []
    for i in range(N_CORES):
        cs = slice(HC * i, HC * (i + 1))
        wk_i = np.ascontiguousarray(
            np.concatenate(
                [w_qkv[:, cs], w_qkv[:, C:][:, cs], w_qkv[:, 2 * C :][:, cs]], axis=1
            )
        ).astype(NPBF16)
        wp_i = np.ascontiguousarray(w_proj[cs, :]).astype(NPBF16)
        in_maps.append(
            {"xT": xT, "wk": wk_i, "wp": wp_i, "zt": zt, "ident": ident}
        )
    return in_maps


_CACHED_NC = None


def kernel(x: np.ndarray, w_qkv: np.ndarray, w_proj: np.ndarray) -> np.ndarray:
    global _CACHED_NC
    if _CACHED_NC is None:
        _CACHED_NC = build_program()
    nc = _CACHED_NC

    in_maps = make_in_maps(x, w_qkv, w_proj)
    res = run_bass_kernel_spmd(nc, in_maps, core_ids=list(range(N_CORES)))
    total = np.zeros((BT, C), dtype=np.float64)
    for i in range(N_CORES):
        total += np.asarray(res.results[i]["po"], dtype=np.float64)
    return total.astype(np.float32).reshape(B, T, C)


if __name__ == "__main__":
    rng = np.random.default_rng(0)
    x = rng.standard_normal((B, T, C), dtype=np.float32)
    w_qkv = rng.standard_normal((C, 3 * C), dtype=np.float32) / np.sqrt(C)
    w_proj = rng.standard_normal((C, C), dtype=np.float32) / np.sqrt(C)
    out = kernel(x=x, w_qkv=w_qkv, w_proj=w_proj)
    print(out.shape, out.dtype, np.abs(out).mean())
